# revision 26
# baseline (speedup 1.0000x reference)
"""MoE expert-parallel MLP kernel for Trainium2 (8 NeuronCores).

Problem: x:(1,8,2048,2048) f32, wi:(8,2048,4096), wo:(8,4096,2048)
         out = gelu_exact(x @ wi) @ wo   (per expert)

Sharding: expert parallelism — core e handles expert e entirely. No
collectives. Per-core math (C=2048 tokens, H=2048 hidden, I=4096 inter):

  GEMM1: h1[I, C] = wi[H, I].T @ xT[H, C]   (lhsT = wi, natural layout)
  gelu:  h1 = gelu(h1)                       (ScalarE, exact erf gelu)
  GEMM2: out[C, H] = h1[I, C].T @ wo[I, H]   (lhsT = h1, natural layout)

Activations and weights are carried in bf16 (wi/wo converted on-chip by
ScalarE as they stream; PSUM accumulation stays fp32, end-to-end rel err
~2e-3 vs the 2e-2 budget). That frees enough SBUF to keep the C-half0
columns of h1 (8 MiB) resident, so only the half1 columns round-trip
through DRAM (16 MiB of h1 traffic instead of 64 MiB) — the
GEMM1->GEMM2 transition has no DRAM dependency at all.

Schedule:
 - Ramp: 4-cb transpose blocks interleaved with io=0..5 matmuls on the
   just-transposed c5 column group (wi_0..5 stay resident), keeping the
   PE busy while x streams in.
 - Steady GEMM1 io-major; the last io finishes its half1 columns first
   so h1res (half0) completes with the final matmul.
 - GEMM2 runs ho-major (each wo quad is loaded+converted once, used by
   both C halves), in 4-bank co sub-groups so PSUM drains (DVE) ping-pong
   under the next sub-group's matmuls. The last sub-group is split into
   co-pairs to shrink the end-of-kernel drain tail.
"""
import numpy as np
from contextlib import ExitStack

import concourse.bass as bass
import concourse.tile as tile
from concourse import bacc, mybir
from concourse.bass_utils import run_bass_kernel_spmd
from concourse.masks import make_identity

P = 128
C, H, I = 2048, 2048, 4096
E = 8
F32 = mybir.dt.float32
F32R = mybir.dt.float32r
BF = mybir.dt.bfloat16

CB = C // P        # 16 C 128-blocks
HB = H // P        # 16 H 128-blocks (K-subtiles of GEMM1)
IB = I // P        # 32 I 128-blocks (K-subtiles of GEMM2)
N5 = 512
C5 = C // N5       # 4
H5 = H // N5       # 4
HALF = C // 2      # 1024
RAMP_IO = 6        # wi rows kept resident through the ramp
GELU = mybir.ActivationFunctionType.Gelu


def _build():
    nc = bacc.Bacc("TRN2", target_bir_lowering=False, debug=False, num_devices=E)
    x = nc.dram_tensor("x", [C, H], F32, kind="ExternalInput").ap()
    wi = nc.dram_tensor("wi", [H, I], F32, kind="ExternalInput").ap()
    wo = nc.dram_tensor("wo", [I, H], F32, kind="ExternalInput").ap()
    out = nc.dram_tensor("out", [C, H], F32, kind="ExternalOutput").ap()

    with tile.TileContext(nc) as tc, ExitStack() as ctx:
        xt_pool = ctx.enter_context(tc.tile_pool(name="xt", bufs=1))
        h1_pool = ctx.enter_context(tc.tile_pool(name="h1res", bufs=1))
        wpool = ctx.enter_context(tc.tile_pool(name="wpool", bufs=8))
        fpool = ctx.enter_context(tc.tile_pool(name="fpool", bufs=3))
        gpool = ctx.enter_context(tc.tile_pool(name="gpool", bufs=4))
        const = ctx.enter_context(tc.tile_pool(name="const", bufs=1))
        psum = ctx.enter_context(tc.tile_pool(name="psum", bufs=8, space="PSUM"))
        dram = ctx.enter_context(tc.tile_pool(name="dram", bufs=1, space="DRAM"))

        h1d = dram.tile([I, HALF], BF)   # C-half1 columns of h1

        ident = const.tile([P, P], F32)
        make_identity(nc, ident[:])
        ident_r = const.tile([P, P], F32R)
        nc.sync.dma_start(ident_r[:], ident[:].bitcast(F32R))

        xT = xt_pool.tile([P, HB, C], BF, tag="big", name="xT")
        h1res = h1_pool.tile([P, IB, HALF], BF, tag="h1", name="h1res")

        wi_tiles = {}

        def _load_wi(io):
            st = fpool.tile([P, HB, P], F32, tag="fs", name=f"wist_{io}")
            nc.sync.dma_start(
                st[:],
                wi[:, io * P:(io + 1) * P].rearrange("(k p) i -> p k i", p=P),
            )
            t = wpool.tile([P, HB, P], BF, tag="w", name=f"wi_{io}")
            nc.scalar.copy(t[:], st[:])
            wi_tiles[io] = t

        def _transpose_cb(cb, split4=False):
            # all x DMAs ride the SP queue: the Act queue must stay free for
            # gelu/copies, whose PSUM drains gate the PE
            x_row = fpool.tile([P, H], F32R, tag="fs", name=f"xrow_{cb}")
            nsplit = 4 if split4 else 2
            step = H // nsplit
            for q in range(nsplit):
                nc.sync.dma_start(
                    x_row[:, q * step:(q + 1) * step],
                    x[cb * P:(cb + 1) * P, q * step:(q + 1) * step].bitcast(F32R),
                )
            for hb4 in range(HB // 4):
                ps_t = psum.tile([P, N5], F32R, tag="mm", name=f"tp_{cb}_{hb4}")
                for j in range(4):
                    hb = hb4 * 4 + j
                    nc.tensor.transpose(
                        ps_t[:, j * P:(j + 1) * P],
                        x_row[:, hb * P:(hb + 1) * P],
                        ident_r[:],
                    )
                dst = xT[:, hb4 * 4:hb4 * 4 + 4, cb * P:(cb + 1) * P]
                src = ps_t[:].bitcast(F32).rearrange("p (j c) -> p j c", j=4)
                # alternate DVE/Act so bank recycling keeps pace with the PE
                if hb4 % 2 == 0:
                    nc.vector.tensor_copy(dst, src)
                else:
                    nc.scalar.copy(dst, src)

        def _xbar_tp(cb, bfpool):
            # XBAR path: f32 row -> Act bf16 convert -> DMA-engine transpose
            # into xT; no PE time. Issued a block ahead of use so the
            # serialized DMA queue absorbs it. bfpool picks where the bf16
            # row stages: wpool has free slots early, but late cbs must use
            # fpool (wpool slots there recycle only after the c5=3 matmuls
            # that need this very transpose - a deadlock).
            x_row = fpool.tile([P, H], F32, tag="fs", name=f"xrow_{cb}")
            for q in range(2):
                nc.sync.dma_start(
                    x_row[:, q * HALF:(q + 1) * HALF],
                    x[cb * P:(cb + 1) * P, q * HALF:(q + 1) * HALF],
                )
            tag = "w" if bfpool is wpool else "fs"
            xbf = bfpool.tile([P, H], BF, tag=tag, name=f"xbf_{cb}")
            nc.scalar.copy(xbf[:], x_row[:])
            nc.scalar.dma_start_transpose(xT[:, :, cb * P:(cb + 1) * P], xbf[:])

        def _mm1(io, c5):
            ps = psum.tile([P, N5], F32, tag="mm", name=f"ps1_{io}_{c5}")
            wt = wi_tiles[io]
            for k in range(HB):
                nc.tensor.matmul(
                    ps[:],
                    wt[:, k, :],
                    xT[:, k, c5 * N5:(c5 + 1) * N5],
                    start=(k == 0),
                    stop=(k == HB - 1),
                )
            return ps

        def _mm1_chunked(io, c5):
            # N=256 halves: the first half only needs the first cb-pair of
            # this c5 group, so blk0's matmuls start two x-rows earlier
            ps = psum.tile([P, N5], F32, tag="mm", name=f"ps1c_{io}_{c5}")
            for h in range(2):
                cols = slice(c5 * N5 + h * 256, c5 * N5 + (h + 1) * 256)
                for k in range(HB):
                    nc.tensor.matmul(
                        ps[:, h * 256:(h + 1) * 256],
                        wi_tiles[io][:, k, :],
                        xT[:, k, cols],
                        start=(k == 0),
                        stop=(k == HB - 1),
                    )
            return ps

        def _drain1(io, c5, ps):
            if c5 < 2:
                # half0 columns: gelu straight into the SBUF-resident h1
                nc.scalar.activation(
                    h1res[:, io, c5 * N5:(c5 + 1) * N5], ps[:], GELU
                )
            else:
                g = gpool.tile([P, N5], BF, tag="g", name=f"g_{io}_{c5}")
                nc.scalar.activation(g[:], ps[:], GELU)
                nc.scalar.dma_start(
                    h1d[io * P:(io + 1) * P, (c5 - 2) * N5:(c5 - 1) * N5], g[:]
                )

        wo_tiles = {}

        def _load_wo(ho, o):
            st = fpool.tile([P, 4, N5], F32, tag="fs", name=f"wost_{ho}_{o}")
            nc.sync.dma_start(
                st[:],
                wo[o * 4 * P:(o + 1) * 4 * P, ho * N5:(ho + 1) * N5]
                .rearrange("(s p) h -> p s h", p=P),
            )
            t = wpool.tile([P, 4, N5], BF, tag="w", name=f"wo_{ho}_{o}")
            nc.scalar.copy(t[:], st[:])
            wo_tiles[(ho, o)] = t

        # ---- Ramp: transpose blocks interleaved with io 0..5 matmuls ----
        # blk 0 interleaves the wi loads with the x rows so neither queue
        # head-blocks the other; wi_4/5 load under the first matmul groups.
        for cb in range(2):
            _transpose_cb(cb, split4=True)
            _load_wi(cb)
        # cb2/cb3's x rows go ahead of wi_2/3 on the queue: the second-half
        # chunks of the first matmul groups need them sooner
        _transpose_cb(2)
        _transpose_cb(3)
        _load_wi(2)
        _load_wi(3)
        for io in range(4):
            _drain1(io, 0, _mm1_chunked(io, 0))
        _load_wi(4)
        _load_wi(5)
        # each block's transposes are interleaved with the previous c5's
        # io=4/5 matmul groups so the PE isn't paced by the x stream
        XBAR_CBS = (10, 11)
        for blk in range(1, 4):
            for cb in (4 * blk, 4 * blk + 1):
                _transpose_cb(cb)
            _drain1(4, blk - 1, _mm1(4, blk - 1))
            for cb in (4 * blk + 2, 4 * blk + 3):
                if cb not in XBAR_CBS:
                    _transpose_cb(cb)
            _drain1(5, blk - 1, _mm1(5, blk - 1))
            if blk == 1:
                # next block's cb 10/11 go through the DMA XBAR
                for cb in XBAR_CBS:
                    _xbar_tp(cb, wpool)
            for io in range(4):
                _drain1(io, blk, _mm1(io, blk))
        for io in (4, 5):
            _drain1(io, 3, _mm1(io, 3))

        # ---- GEMM1 steady: io-major over the remaining rows ----
        for io in range(RAMP_IO, IB):
            if io not in wi_tiles:
                _load_wi(io)
            if io + 1 < IB and (io + 1) not in wi_tiles:
                _load_wi(io + 1)
            # weave the first ho=0 wo quads into the last few io rows
            # (not io=31 — its gelu must not queue behind wo converts)
            if IB - 5 <= io <= IB - 2:
                o0 = 2 * (io - (IB - 5))
                _load_wo(0, o0)
                _load_wo(0, o0 + 1)
            # the final row finishes its DRAM-bound half first so h1res
            # (and with it GEMM2) unblocks with the very last matmul
            order = (2, 3, 0, 1) if io == IB - 1 else (0, 1, 2, 3)
            for c5 in order:
                _drain1(io, c5, _mm1(io, c5))
            wi_tiles.pop(io)

        # ---- GEMM2: out = h1.T @ wo, ho-major, 4-bank co sub-groups ----
        h1r1 = xt_pool.tile([P, IB, HALF], BF, tag="big", name="h1r1")
        for ik in range(IB):
            nc.sync.dma_start(h1r1[:, ik, :], h1d[ik * P:(ik + 1) * P, :])

        def _mm2_group(ho, half, cos, lhs, n0=0, n1=N5):
            w = n1 - n0
            pss = [
                psum.tile([P, w], F32, tag="mm", name=f"ps2_{ho}_{half}_{co}_{n0}")
                for co in cos
            ]
            for ik in range(IB):
                wo_t = wo_tiles[(ho, ik // 4)]
                for i, co in enumerate(cos):
                    nc.tensor.matmul(
                        pss[i][:],
                        lhs[:, ik, co * P:(co + 1) * P],
                        wo_t[:, ik % 4, n0:n1],
                        start=(ik == 0),
                        stop=(ik == IB - 1),
                    )
            outs = fpool.tile(
                [P, len(cos), w], F32, tag="fs",
                name=f"outs_{ho}_{half}_{cos[0]}_{n0}"
            )
            for i, co in enumerate(cos):
                nc.vector.tensor_copy(outs[:, i, :], pss[i][:])
            r0 = half * HALF + cos[0] * P
            out_dst = (
                out[r0:r0 + len(cos) * P, ho * N5 + n0:ho * N5 + n1]
                .rearrange("(co p) h -> p co h", p=P)
            )
            nc.scalar.dma_start(out_dst, outs[:])

        for ho in range(H5):
            for half in range(2):
                lhs = h1res if half == 0 else h1r1
                if ho == H5 - 1 and half == 1:
                    # shrinking final groups so the last drain+store is tiny
                    for cos in ((0, 1, 2, 3), (4, 5), (6,)):
                        _mm2_group(ho, half, cos, lhs)
                    _mm2_group(ho, half, (7,), lhs, 0, 256)
                    _mm2_group(ho, half, (7,), lhs, 256, N5)
                else:
                    _mm2_group(ho, half, (0, 1, 2, 3), lhs)
                    if half == 1 and ho + 1 < H5:
                        # prefetch next ho's quads as this ho's slots free up
                        for o in range(4):
                            _load_wo(ho + 1, o)
                    _mm2_group(ho, half, (4, 5, 6, 7), lhs)
                    if half == 1 and ho + 1 < H5:
                        for o in range(4, 8):
                            _load_wo(ho + 1, o)
            for o in range(8):
                wo_tiles.pop((ho, o))

    nc.compile()
    return nc


_NC = None


def kernel(x, wi, wo):
    global _NC
    if _NC is None:
        _NC = _build()
    x = np.ascontiguousarray(np.asarray(x, dtype=np.float32)).reshape(E, C, H)
    wi = np.ascontiguousarray(np.asarray(wi, dtype=np.float32))
    wo = np.ascontiguousarray(np.asarray(wo, dtype=np.float32))
    in_maps = [
        {"x": x[e], "wi": wi[e], "wo": wo[e]}
        for e in range(E)
    ]
    res = run_bass_kernel_spmd(_NC, in_maps, core_ids=list(range(E)))
    out = np.stack([res.results[e]["out"] for e in range(E)])[None]
    return out
